# revision 2
# baseline (speedup 1.0000x reference)
"""ChebyKANLinear Trainium2 kernel.

Math: y[b,o] = (1/I) * sum_{i,d} T_d(c[b,i]) * W[i,o,d],  c = clip(tanh(x), -1+eps, 1-eps)
with Chebyshev T_0=1, T_1=c, T_2=2c^2-1, T_3=4c^3-3c.

Re-expressed in the monomial basis (exact linear recombination, folded into the
weights on the host):
    y = bias + c @ V1 + c^2 @ V2 + c^3 @ V3
    V1 = (W1 - 3*W3)/I, V2 = 2*W2/I, V3 = 4*W3/I, bias_o = sum_i (W0 - W2)[i,o]/I

Sharding: data-parallel over the batch dim across 8 NeuronCores (x transposed on
host so the contraction dim i lands on SBUF partitions); V/bias replicated.
On-device per core: tanh (ACT), clip/square/cube (DVE), then 2x7 accumulating
matmuls into PSUM (a K=1 ones-matmul broadcasts the bias), scale-free copy out.
"""

from contextlib import ExitStack

import numpy as np

import concourse.bass as bass
import concourse.tile as tile
from concourse import bacc, mybir
from concourse.bass_utils import run_bass_kernel_spmd

N_CORES = 8
B, I, O, D = 2048, 256, 256, 4
BL = B // N_CORES  # batch rows per core
EPS = 1e-07
F32 = mybir.dt.float32

_cache = {}


def _build_program():
    nc = bacc.Bacc("TRN2", target_bir_lowering=False, debug=False, num_devices=N_CORES)

    # [i_half, i_in_half, b_local]  (x slice pre-transposed on host)
    xt_d = nc.dram_tensor("xt", [2, 128, BL], F32, kind="ExternalInput")
    # [d-1, i_half, i_in_half, o]
    v_d = nc.dram_tensor("v", [3, 2, 128, O], F32, kind="ExternalInput")
    bias_d = nc.dram_tensor("bias", [1, O], F32, kind="ExternalInput")
    # [b_half, b_in_half, o]
    y_d = nc.dram_tensor("y", [BL // 128, 128, O], F32, kind="ExternalOutput")

    # python floats holding the exact f32-rounded clip bounds
    clip_lo = float(np.float32(-1.0 + EPS))
    clip_hi = float(np.float32(1.0 - EPS))

    with tile.TileContext(nc) as tc, ExitStack() as ctx:
        pool = ctx.enter_context(tc.tile_pool(name="main", bufs=1))
        psum = ctx.enter_context(
            tc.tile_pool(name="psum", bufs=1, space=bass.MemorySpace.PSUM)
        )

        ones = pool.tile([1, 128], F32, tag="ones")
        nc.vector.memset(ones[:], 1.0)
        bias_t = pool.tile([1, O], F32, tag="bias")
        nc.sync.dma_start(bias_t[:], bias_d[:])

        vt = {}
        for d in range(3):
            for ih in range(2):
                t = pool.tile([128, O], F32, tag=f"v{d}{ih}")
                nc.sync.dma_start(t[:], v_d[d, ih])
                vt[(d, ih)] = t

        basis = {}
        for ih in range(2):
            xt = pool.tile([128, BL], F32, tag=f"xt{ih}")
            nc.sync.dma_start(xt[:], xt_d[ih])
            c = pool.tile([128, BL], F32, tag=f"c{ih}")
            nc.scalar.activation(c[:], xt[:], mybir.ActivationFunctionType.Tanh)
            # in-place clip: c = min(max(c, lo), hi)
            nc.vector.tensor_scalar(
                c[:], c[:], clip_lo, clip_hi, mybir.AluOpType.max, mybir.AluOpType.min
            )
            c2 = pool.tile([128, BL], F32, tag=f"c2{ih}")
            nc.scalar.activation(c2[:], c[:], mybir.ActivationFunctionType.Square)
            c3 = pool.tile([128, BL], F32, tag=f"c3{ih}")
            nc.vector.tensor_mul(c3[:], c2[:], c[:])
            basis[(0, ih)] = c
            basis[(1, ih)] = c2
            basis[(2, ih)] = c3

        for bh in range(BL // 128):
            acc = psum.tile([128, O], F32, tag=f"acc{bh}")
            # bias broadcast: [128,1] ones.T @ [1,O] bias
            nc.tensor.matmul(acc[:], ones[:], bias_t[:], start=True, stop=False)
            for d in range(3):
                for ih in range(2):
                    nc.tensor.matmul(
                        acc[:],
                        basis[(d, ih)][:, bh * 128 : (bh + 1) * 128],
                        vt[(d, ih)][:],
                        start=False,
                        stop=(d == 2 and ih == 1),
                    )
            y_sb = pool.tile([128, O], F32, tag=f"y{bh}")
            nc.vector.tensor_copy(y_sb[:], acc[:])
            nc.sync.dma_start(y_d[bh], y_sb[:])

    nc.compile()
    return nc


def _get_program():
    if "nc" not in _cache:
        _cache["nc"] = _build_program()
    return _cache["nc"]


def kernel(x, cheby_coeffs):
    x = np.ascontiguousarray(x, dtype=np.float32)
    W = np.ascontiguousarray(cheby_coeffs, dtype=np.float32)
    assert x.shape == (B, I) and W.shape == (I, O, D)

    inv_i = np.float32(1.0 / I)
    V = np.stack(
        [
            W[:, :, 1] - 3.0 * W[:, :, 3],
            2.0 * W[:, :, 2],
            4.0 * W[:, :, 3],
        ]
    ).astype(np.float32) * inv_i  # [3, I, O]
    v_arr = np.ascontiguousarray(V.reshape(3, 2, 128, O))
    bias = ((W[:, :, 0] - W[:, :, 2]).sum(axis=0, dtype=np.float32) * inv_i).reshape(
        1, O
    )
    bias = np.ascontiguousarray(bias, dtype=np.float32)

    nc = _get_program()
    in_maps = []
    for c_id in range(N_CORES):
        xs = x[c_id * BL : (c_id + 1) * BL, :]  # [BL, I]
        xt = np.ascontiguousarray(xs.T).reshape(2, 128, BL)
        in_maps.append({"xt": xt, "v": v_arr, "bias": bias})

    res = run_bass_kernel_spmd(nc, in_maps, list(range(N_CORES)))
    y = np.concatenate(
        [r["y"].reshape(BL, O) for r in res.results], axis=0
    )
    return y


# revision 5
# speedup vs baseline: 1.1262x; 1.1262x over previous
"""ChebyKANLinear Trainium2 kernel.

Math: y[b,o] = (1/I) * sum_{i,d} T_d(c[b,i]) * W[i,o,d],  c = clip(tanh(x), -1+eps, 1-eps)
with Chebyshev T_0=1, T_1=c, T_2=2c^2-1, T_3=4c^3-3c.

Re-expressed in the monomial basis (exact linear recombination, folded into the
weights on the host):
    y = bias + c @ V1 + c^2 @ V2 + c^3 @ V3
    V1 = (W1 - 3*W3)/I, V2 = 2*W2/I, V3 = 4*W3/I, bias_o = sum_i (W0 - W2)[i,o]/I

Sharding: data-parallel over the batch dim across 8 NeuronCores (x transposed on
host so the contraction dim i lands on SBUF partitions); V/bias replicated.
On-device per core: tanh (ACT), clip/square/cube (DVE), then 2x7 accumulating
matmuls into PSUM (a K=1 ones-matmul broadcasts the bias), scale-free copy out.
"""

from contextlib import ExitStack

import numpy as np

import concourse.bass as bass
import concourse.tile as tile
from concourse import bacc, mybir
from concourse.bass_utils import run_bass_kernel_spmd

N_CORES = 8
B, I, O, D = 2048, 256, 256, 4
BL = B // N_CORES  # batch rows per core
EPS = 1e-07
F32 = mybir.dt.float32

_cache = {}


def _build_program():
    nc = bacc.Bacc("TRN2", target_bir_lowering=False, debug=False, num_devices=N_CORES)

    # [i_half, i_in_half, b_local]  (x slice pre-transposed on host)
    xt_d = nc.dram_tensor("xt", [2, 128, BL], F32, kind="ExternalInput")
    # [d-1, i_half, i_in_half, o]
    v_d = nc.dram_tensor("v", [3, 2, 128, O], F32, kind="ExternalInput")
    bias_d = nc.dram_tensor("bias", [1, O], F32, kind="ExternalInput")
    # [b_half, b_in_half, o]
    y_d = nc.dram_tensor("y", [BL // 128, 128, O], F32, kind="ExternalOutput")

    # python floats holding the exact f32-rounded clip bounds
    clip_lo = float(np.float32(-1.0 + EPS))
    clip_hi = float(np.float32(1.0 - EPS))

    with tile.TileContext(nc) as tc, ExitStack() as ctx:
        pool = ctx.enter_context(tc.tile_pool(name="main", bufs=1))
        psum = ctx.enter_context(
            tc.tile_pool(name="psum", bufs=1, space=bass.MemorySpace.PSUM)
        )

        # x tiles first — the tanh->clip->c2->c3 chain is the longest dep path
        xt = {}
        for ih in range(2):
            xt[ih] = pool.tile([128, BL], F32, tag=f"xt{ih}", name=f"xt{ih}")
        nc.sync.dma_start(xt[0][:], xt_d[0])
        nc.scalar.dma_start(xt[1][:], xt_d[1])

        ones = pool.tile([1, 128], F32, tag="ones")
        nc.gpsimd.memset(ones[:], 1.0)
        bias_t = pool.tile([1, O], F32, tag="bias")
        nc.gpsimd.dma_start(bias_t[:], bias_d[:])

        # V tiles spread across engine HWDGE queues so they don't serialize
        vt = {}
        for d in range(3):
            for ih in range(2):
                vt[(d, ih)] = pool.tile([128, O], F32, tag=f"v{d}{ih}", name=f"v{d}{ih}")
        nc.gpsimd.dma_start(vt[(0, 0)][:], v_d[0, 0])
        nc.sync.dma_start(vt[(0, 1)][:], v_d[0, 1])
        nc.scalar.dma_start(vt[(1, 0)][:], v_d[1, 0])
        nc.gpsimd.dma_start(vt[(1, 1)][:], v_d[1, 1])
        nc.sync.dma_start(vt[(2, 0)][:], v_d[2, 0])
        nc.scalar.dma_start(vt[(2, 1)][:], v_d[2, 1])

        basis = {}
        for ih in range(2):
            c = pool.tile([128, BL], F32, tag=f"c{ih}")
            nc.scalar.activation(c[:], xt[ih][:], mybir.ActivationFunctionType.Tanh)
            # in-place clip: c = min(max(c, lo), hi)  (single fused DVE op)
            nc.vector.tensor_scalar(
                c[:], c[:], clip_lo, clip_hi, mybir.AluOpType.max, mybir.AluOpType.min
            )
            c2 = pool.tile([128, BL], F32, tag=f"c2{ih}")
            nc.vector.tensor_mul(c2[:], c[:], c[:])
            c3 = pool.tile([128, BL], F32, tag=f"c3{ih}")
            nc.vector.tensor_mul(c3[:], c2[:], c[:])
            basis[(0, ih)] = c
            basis[(1, ih)] = c2
            basis[(2, ih)] = c3

        n_bh = BL // 128
        acc = {}
        for bh in range(n_bh):
            acc[bh] = psum.tile([128, O], F32, tag=f"acc{bh}", name=f"acc{bh}")
            # bias broadcast: [128,1] ones.T @ [1,O] bias
            nc.tensor.matmul(acc[bh][:], ones[:], bias_t[:], start=True, stop=False)
        # interleave bh so each arriving operand pair unblocks two matmuls
        for d in range(3):
            for ih in range(2):
                for bh in range(n_bh):
                    nc.tensor.matmul(
                        acc[bh][:],
                        basis[(d, ih)][:, bh * 128 : (bh + 1) * 128],
                        vt[(d, ih)][:],
                        start=False,
                        stop=(d == 2 and ih == 1),
                    )
        for bh in range(n_bh):
            y_sb = pool.tile([128, O], F32, tag=f"y{bh}")
            nc.vector.tensor_copy(y_sb[:], acc[bh][:])
            (nc.sync if bh == 0 else nc.scalar).dma_start(y_d[bh], y_sb[:])

    nc.compile()
    return nc


def _get_program():
    if "nc" not in _cache:
        _cache["nc"] = _build_program()
    return _cache["nc"]


def kernel(x, cheby_coeffs):
    x = np.ascontiguousarray(x, dtype=np.float32)
    W = np.ascontiguousarray(cheby_coeffs, dtype=np.float32)
    assert x.shape == (B, I) and W.shape == (I, O, D)

    inv_i = np.float32(1.0 / I)
    V = np.stack(
        [
            W[:, :, 1] - 3.0 * W[:, :, 3],
            2.0 * W[:, :, 2],
            4.0 * W[:, :, 3],
        ]
    ).astype(np.float32) * inv_i  # [3, I, O]
    v_arr = np.ascontiguousarray(V.reshape(3, 2, 128, O))
    bias = ((W[:, :, 0] - W[:, :, 2]).sum(axis=0, dtype=np.float32) * inv_i).reshape(
        1, O
    )
    bias = np.ascontiguousarray(bias, dtype=np.float32)

    nc = _get_program()
    in_maps = []
    for c_id in range(N_CORES):
        xs = x[c_id * BL : (c_id + 1) * BL, :]  # [BL, I]
        xt = np.ascontiguousarray(xs.T).reshape(2, 128, BL)
        in_maps.append({"xt": xt, "v": v_arr, "bias": bias})

    res = run_bass_kernel_spmd(nc, in_maps, list(range(N_CORES)))
    y = np.concatenate(
        [r["y"].reshape(BL, O) for r in res.results], axis=0
    )
    return y
